# revision 25
# baseline (speedup 1.0000x reference)
"""Distributed causal GQA attention kernel for one TRN2 chip (8 NeuronCores).

Problem: b=1, T=2048, d_model=2048, 32 Q heads, 8 KV heads, head_dim=64,
llama3-scaled RoPE, causal softmax attention, out-projection.

Sharding (tensor-parallel over heads):
  core i holds Q heads 4i..4i+3 and KV head i (GQA groups align exactly),
  plus Wo rows 256i..256(i+1) (i.e. output-column shard).
Pipeline per core:
  - QKV projections from pre-transposed x (fp32r matmuls, ~11-bit mantissa)
  - RoPE applied in transposed [d, t] layout with host-built sign-folded tables
  - attention computed as S^T = K^T(kblock)ᵀ·Q^T per 128-k-block x 512-q-chunk,
    causal blocks skipped, diagonal blocks masked; exp on ScalarE with the
    1/sqrt(64) scale folded in; PV matmul with ones-augmented V so the softmax
    denominator falls out of the same matmul; VectorE reciprocal to normalize.
  - context (bf16) AllGather across the 8 cores, chunked by q so the collective
    overlaps attention/out-projection of other chunks.
  - out-projection (bf16) against the Wo column shard; each core writes a
    disjoint [256, 2048] slice of out^T; the host concatenates (no AllReduce).
"""

import sys

for _p in ("/opt/trn_rl_repo", "/root/.axon_site/_ro/trn_rl_repo"):
    if _p not in sys.path:
        sys.path.append(_p)

import numpy as np

import concourse.bass as bass
import concourse.bacc as bacc
import concourse.tile as tile
import concourse.mybir as mybir
from concourse.bass_utils import run_bass_kernel_spmd

F32 = mybir.dt.float32
F32R = mybir.dt.float32r
BF16 = mybir.dt.bfloat16
EXP = mybir.ActivationFunctionType.Exp
LOG = mybir.ActivationFunctionType.Ln

N_CORES = 8
T = 2048          # sequence length
D = 2048          # model dim
HD = 64           # head dim
HPC = 4           # q heads per core
DLOC = HPC * HD   # 256 local q-head dims / Wo rows per core
QCW = 512         # q chunk width
NQC = T // QCW    # 4
KBW = 128         # k block width
NKB = T // KBW    # 16
NM = D // 128     # 16 contraction chunks
ROPE_BASE = 500000.0
FREQ_CONFIG = {"factor": 32.0, "low_freq_factor": 1.0, "high_freq_factor": 4.0,
               "original_context_length": 8192}


def _round_fp32r(a: np.ndarray) -> np.ndarray:
    """Round fp32 to the fp32r format (1/8/11 float in the top 20 bits, RNE)."""
    u = np.ascontiguousarray(a, dtype=np.float32).view(np.uint32)
    bias = ((u >> np.uint32(12)) & np.uint32(1)) + np.uint32(0x7FF)
    u = (u + bias) & np.uint32(0xFFFFF000)
    return u.view(np.float32)


def _rope_tables(start_pos: int):
    fc = FREQ_CONFIG
    inv_freq = 1.0 / ROPE_BASE ** (np.arange(0, HD, 2, dtype=np.float32) / HD)
    low_wl = fc["original_context_length"] / fc["low_freq_factor"]
    high_wl = fc["original_context_length"] / fc["high_freq_factor"]
    wavelen = 2.0 * np.pi / inv_freq
    inv_l = np.where(wavelen > low_wl, inv_freq / fc["factor"], inv_freq)
    smooth = (fc["original_context_length"] / wavelen - fc["low_freq_factor"]) / (
        fc["high_freq_factor"] - fc["low_freq_factor"])
    smoothed = (1.0 - smooth) * (inv_freq / fc["factor"]) + smooth * inv_freq
    med = (wavelen <= low_wl) & (wavelen >= high_wl)
    inv_freq = np.where(med, smoothed, inv_l)
    pos = np.arange(start_pos, start_pos + T, dtype=np.float32)
    ang = pos[:, None] * inv_freq[None, :]
    ang = np.concatenate([ang, ang], axis=1)          # [T, 64]
    cos = np.cos(ang).astype(np.float32)
    sin = np.sin(ang).astype(np.float32)
    cosT = cos.T                                       # [64, T]
    # fold the rotate-half sign into sin: rows 0..31 multiply -x2, rows 32..63 +x1
    sinT_signed = np.concatenate([-sin[:, :32].T, sin[:, 32:].T], axis=0)
    return (np.ascontiguousarray(np.tile(cosT, (2, 1))),
            np.ascontiguousarray(np.tile(sinT_signed, (2, 1))))  # [128, T] each


def _patch_activation_tables():
    """Make Exp and Ln resolve only to the combined natural_log_exp set so
    the table-load pass emits one load instead of thrashing between the
    exp-only and ln-only sets (2.7us per switch, mid-attention)."""
    import functools
    from concourse.hw_specs import get_activation_tables as orig

    @functools.cache
    def patched(arch):
        tables = dict(orig(arch))
        comb = "natural_log_exp_and_others"
        if comb not in tables:
            return tables
        exp_ln = {mybir.ActivationFunctionType.Exp,
                  mybir.ActivationFunctionType.Ln}
        return {name: (funcs if name == comb else funcs - exp_ln)
                for name, funcs in tables.items()}

    bacc.get_activation_tables = patched


def build_nc():
    _patch_activation_tables()
    nc = bacc.Bacc("TRN2", target_bir_lowering=False, debug=False,
                   num_devices=N_CORES)

    xT = nc.dram_tensor("xT", [D, T], F32R, kind="ExternalInput")
    wqT = nc.dram_tensor("wqT", [D, DLOC], F32R, kind="ExternalInput")
    wkvT = nc.dram_tensor("wkvT", [D, 128], F32R, kind="ExternalInput")
    woT = nc.dram_tensor("woT", [D, DLOC], BF16, kind="ExternalInput")
    cosT = nc.dram_tensor("cosT", [128, T], F32, kind="ExternalInput")
    sinT = nc.dram_tensor("sinT", [128, T], F32, kind="ExternalInput")
    mask4 = nc.dram_tensor("mask4", [128, 4 * KBW], F32, kind="ExternalInput")
    eye64 = nc.dram_tensor("eye64", [128, 64], F32, kind="ExternalInput")
    out = nc.dram_tensor("out", [DLOC, T], F32, kind="ExternalOutput")

    rg = [list(range(N_CORES))]

    with tile.TileContext(nc) as tc:
        with tc.tile_pool(name="wpool", bufs=1) as wpool, \
             tc.tile_pool(name="xpool", bufs=10) as xpool, \
             tc.tile_pool(name="qpool", bufs=1) as qpool, \
             tc.tile_pool(name="ppool", bufs=3) as ppool, \
             tc.tile_pool(name="npool", bufs=2) as npool, \
             tc.tile_pool(name="agpool", bufs=2) as agpool, \
             tc.tile_pool(name="dram", bufs=1, space="DRAM") as dram:

            # ---- resident constants / weights ----
            # weights first (the first matmuls wait on them), split into
            # 4-m-chunk pieces so matmul m=0 isn't gated on the full 2MB
            cos_sb = wpool.tile([128, T], F32)
            sin_sb = wpool.tile([128, T], F32)
            mask_sb = wpool.tile([128, 4 * KBW], F32)
            eye_sb = wpool.tile([128, 64], F32)
            wq_sb = wpool.tile([128, NM, DLOC], F32R)
            wkv_sb = wpool.tile([128, NM, 128], F32R)
            wo_sb = wpool.tile([128, NM, DLOC], BF16)
            for g in range(4):
                sl = slice(4 * g, 4 * g + 4)
                nc.sync.dma_start(out=wq_sb[:, sl, :],
                                  in_=wqT.rearrange("(m p) d -> p m d", p=128)[:, sl, :])
                nc.sync.dma_start(out=wkv_sb[:, sl, :],
                                  in_=wkvT.rearrange("(m p) d -> p m d", p=128)[:, sl, :])
            nc.sync.dma_start(out=eye_sb[:], in_=eye64[:])
            nc.sync.dma_start(out=mask_sb[:], in_=mask4[:])

            # warm the ACT table set early (Log+Exp together so the combined
            # set is resident before the attention phase)
            warm_sb = wpool.tile([1, 16], F32)
            nc.scalar.activation(warm_sb[0:1, :], eye_sb[0:1, 0:16], LOG,
                                 bias=1.0, scale=1.0)
            nc.scalar.activation(warm_sb[0:1, :], eye_sb[0:1, 0:16], EXP,
                                 scale=0.125)

            # ---- resident activations ----
            q_sb0 = qpool.tile([128, T], F32)    # pre-rope Q^T heads 0,1
            q_sb1 = qpool.tile([128, T], F32)    # pre-rope Q^T heads 2,3
            kv_sb = qpool.tile([128, T], F32)    # rows 0:64 K^T, 64:128 V^T
            qr0 = qpool.tile([128, T], F32R)     # rope'd Q^T heads 0,1
            qr1 = qpool.tile([128, T], F32R)     # rope'd Q^T heads 2,3
            kdup = qpool.tile([128, T], F32R)    # rope'd K^T duplicated on both halves
            vprime = [qpool.tile([128, 72], BF16, name=f"vp{b}") for b in range(NKB)]

            # ---- phase 1: QKV projections (x streamed once, n-sliced) ----
            with tc.tile_pool(name="projps", bufs=6, space="PSUM") as projps, \
                 tc.tile_pool(name="tps", bufs=2, space="PSUM") as tps:
                for n in range(NQC):
                    ql = QCW * n
                    qps0 = projps.tile([128, QCW], F32, tag="proj", name=f"qps0_{n}")
                    qps1 = projps.tile([128, QCW], F32, tag="proj", name=f"qps1_{n}")
                    kvps = projps.tile([128, QCW], F32, tag="proj", name=f"kvps_{n}")
                    dma_eng = [nc.sync, nc.scalar]
                    for g in range(NM // 2):
                        xg = xpool.tile([128, 2, QCW], F32R, tag="xg",
                                        name=f"xg_{n}_{g}")
                        dma_eng[(n * 8 + g) % 2].dma_start(
                            out=xg[:],
                            in_=xT.rearrange("(m p) t -> p m t", p=128)[
                                :, 2 * g:2 * g + 2, ql:ql + QCW])
                        for mm in range(2):
                            m = 2 * g + mm
                            first, last = (m == 0), (m == NM - 1)
                            nc.tensor.matmul(kvps[:], wkv_sb[:, m, :],
                                             xg[:, mm, :], start=first, stop=last)
                            nc.tensor.matmul(qps0[:], wq_sb[:, m, 0:128],
                                             xg[:, mm, :], start=first, stop=last)
                            nc.tensor.matmul(qps1[:], wq_sb[:, m, 128:256],
                                             xg[:, mm, :], start=first, stop=last)
                    nc.vector.tensor_copy(kv_sb[:, ql:ql + QCW], kvps[:])
                    nc.vector.tensor_copy(q_sb0[:, ql:ql + QCW], qps0[:])
                    nc.vector.tensor_copy(q_sb1[:, ql:ql + QCW], qps1[:])
                    # V natural layout for this n-slice via PE transpose
                    for bb in range(4):
                        b = 4 * n + bb
                        vt_ps = tps.tile([128, 64], F32, tag="vt",
                                         name=f"vtps_{b}")
                        nc.tensor.transpose(vt_ps[:],
                                            kv_sb[64:128, KBW * b:KBW * (b + 1)],
                                            eye_sb[64:128, :])
                        nc.vector.tensor_copy(vprime[b][:, 0:64], vt_ps[:])
                        # two bf16 1.0 ones columns (even stationary M=66)
                        nc.vector.memset(
                            vprime[b][:, 64:66].bitcast(mybir.dt.uint16),
                            0x3F80)
                # constants needed later: behind the x stream on purpose
                nc.sync.dma_start(out=cos_sb[:], in_=cosT[:])
                nc.sync.dma_start(out=sin_sb[:], in_=sinT[:])
                for g in range(4):
                    sl = slice(4 * g, 4 * g + 4)
                    nc.sync.dma_start(
                        out=wo_sb[:, sl, :],
                        in_=woT.rearrange("(m p) d -> p m d", p=128)[:, sl, :])

            # ---- phase 2: RoPE ----
            rot0 = qpool.tile([128, T], F32, tag="rot", name="rot0")
            for h in range(2):
                nc.sync.dma_start(out=rot0[64 * h:64 * h + 32, :],
                                  in_=q_sb0[64 * h + 32:64 * h + 64, :])
                nc.sync.dma_start(out=rot0[64 * h + 32:64 * h + 64, :],
                                  in_=q_sb0[64 * h:64 * h + 32, :])
            nc.vector.tensor_mul(rot0[:], rot0[:], sin_sb[:])
            nc.vector.tensor_mul(q_sb0[:], q_sb0[:], cos_sb[:])
            nc.vector.tensor_add(qr0[:], q_sb0[:], rot0[:])

            rot1 = qpool.tile([128, T], F32, tag="rot", name="rot1")
            for h in range(2):
                nc.sync.dma_start(out=rot1[64 * h:64 * h + 32, :],
                                  in_=q_sb1[64 * h + 32:64 * h + 64, :])
                nc.sync.dma_start(out=rot1[64 * h + 32:64 * h + 64, :],
                                  in_=q_sb1[64 * h:64 * h + 32, :])
            nc.vector.tensor_mul(rot1[:], rot1[:], sin_sb[:])
            nc.vector.tensor_mul(q_sb1[:], q_sb1[:], cos_sb[:])
            nc.vector.tensor_add(qr1[:], q_sb1[:], rot1[:])

            rotk = qpool.tile([128, T], F32, tag="rot", name="rotk")
            nc.sync.dma_start(out=rotk[0:32, :], in_=kv_sb[32:64, :])
            nc.sync.dma_start(out=rotk[32:64, :], in_=kv_sb[0:32, :])
            nc.vector.tensor_mul(rotk[0:64, :], rotk[0:64, :], sin_sb[0:64, :])
            nc.vector.tensor_mul(kv_sb[0:64, :], kv_sb[0:64, :], cos_sb[0:64, :])
            nc.vector.tensor_add(kdup[0:64, :], kv_sb[0:64, :], rotk[0:64, :])
            nc.sync.dma_start(out=kdup[64:128, :], in_=kdup[0:64, :])

            # ---- phase 3: attention + allgather + out-projection, per q-chunk
            # and head pair; pair p of core r carries j-chunks m = 2r+p ----
            ag_in = {}
            ag_out = {}
            for c in range(NQC):
                for p in range(2):
                    ag_in[c, p] = dram.tile([128, QCW], BF16,
                                            name=f"ag_in_{c}_{p}")
                    ag_out[c, p] = dram.tile([1024, QCW], BF16,
                                             addr_space="Shared",
                                             name=f"ag_out_{c}_{p}")

            # PSUM budget (8 banks): spair [128,1024] bufs=2 -> 4 banks,
            # ctx accumulators [128,512] bufs=2 -> 2, out-proj [128,512]
            # bufs=2 -> 2. Separate tags/pools so attention(qc+1) never
            # waits on out-projection(qc)'s banks, and S double-buffers
            # against the exp.
            with tc.tile_pool(name="spsum", bufs=2, space="PSUM") as spsum, \
                 tc.tile_pool(name="cpsum", bufs=2, space="PSUM") as cpsum, \
                 tc.tile_pool(name="opsum", bufs=2, space="PSUM") as opsum:

                def qr_rhs(h, lo, hi):
                    t_ = qr0 if h < 2 else qr1
                    base = 64 * (h % 2)
                    return t_[base:base + 64, lo:hi]

                def k_lhs(h, b):
                    base = 64 * (h % 2)
                    return kdup[base:base + 64, KBW * b:KBW * (b + 1)]

                outproj_state = {}

                def emit_outproj_part(c, p):
                    if c not in outproj_state:
                        outproj_state[c] = [
                            opsum.tile([128, QCW], F32, tag="ops",
                                       name=f"ops_{c}_{cb}") for cb in range(2)]
                    ops = outproj_state[c]
                    for gg in range(2):
                        agt = agpool.tile([128, 4, QCW], BF16, tag="ag",
                                          name=f"agt_{c}_{p}_{gg}")
                        nc.sync.dma_start(
                            out=agt[:],
                            in_=ag_out[c, p].rearrange(
                                "(g q) t -> q g t", q=128)[:, 4 * gg:4 * gg + 4, :])
                        for gl in range(4):
                            g = 4 * gg + gl
                            m = 2 * g + p
                            for cb in range(2):
                                nc.tensor.matmul(
                                    ops[cb][:],
                                    wo_sb[:, m, 128 * cb:128 * (cb + 1)],
                                    agt[:, gl, :],
                                    start=(p == 0 and g == 0),
                                    stop=(p == 1 and g == 7))

                def emit_outproj_finish(c):
                    ops = outproj_state.pop(c)
                    for cb in range(2):
                        osb = npool.tile([128, QCW], F32, tag="osb",
                                         name=f"osb_{c}_{cb}")
                        nc.vector.tensor_copy(osb[:], ops[cb][:])
                        nc.gpsimd.dma_start(
                            out=out[128 * cb:128 * (cb + 1),
                                    QCW * c:QCW * (c + 1)],
                            in_=osb[:])

                # initialize the two S slots once so the full-tile exp never
                # reads uninitialized PSUM (junk strips are unread downstream)
                for z in range(2):
                    spz = spsum.tile([128, 2 * QCW], F32, tag="spair",
                                     name=f"sp_init_{z}")
                    nc.vector.memset(spz[:], 0.0)

                for c in range(NQC):
                    ql = QCW * c
                    # one head pair at a time through the whole q-chunk
                    for p in range(2):
                        ctx2 = [cpsum.tile([128, QCW], F32, tag="cps",
                                           name=f"ctxp_{c}_{p}_{hh}")
                                for hh in range(2)]

                        def attend_block(b, qs, masked):
                            first = (b == 0)
                            last = (b == 4 * c + 3)
                            sp = spsum.tile([128, 2 * QCW], F32, tag="spair",
                                            name=f"sp_{c}_{p}_{b}")
                            for hh in range(2):
                                h = 2 * p + hh
                                nc.tensor.matmul(
                                    sp[:, QCW * hh + qs:QCW * (hh + 1)],
                                    k_lhs(h, b), qr_rhs(h, ql + qs, ql + QCW),
                                    start=True, stop=True)
                            if masked:
                                sv = sp[:].rearrange("p (h q) -> p h q", h=2)[
                                    :, :, qs:qs + KBW]
                                mv = mask_sb[:, 0:2 * KBW].rearrange(
                                    "p (h q) -> p h q", h=2)
                                nc.vector.tensor_add(sv, sv, mv)
                            if qs > 0:
                                # zero the unwritten strips so the full-tile
                                # exp below reads only this tile's data
                                nc.vector.memset(
                                    sp[:].rearrange("p (h q) -> p h q", h=2)[
                                        :, :, 0:qs], 0.0)
                            pp = ppool.tile([128, 2 * QCW], BF16, tag="pp",
                                            name=f"pp_{c}_{p}_{b}")
                            # always a full contiguous exp; the [0:qs) strip
                            # per head is junk but never read by the PV rhs
                            nc.scalar.activation(pp[:], sp[:], EXP, scale=0.125)
                            for hh in range(2):
                                nc.tensor.matmul(
                                    ctx2[hh][0:66, qs:QCW], vprime[b][:, 0:66],
                                    pp[:, QCW * hh + qs:QCW * (hh + 1)],
                                    start=first, stop=last)

                        for b in range(4 * c):
                            attend_block(b, 0, masked=False)
                        for j in range(4):
                            attend_block(4 * c + j, KBW * j, masked=True)

                        # normalize (1/s = exp(-ln(s)) on ScalarE) + ship
                        for hh in range(2):
                            h = 2 * p + hh
                            lt = npool.tile([1, QCW], F32, tag="lt",
                                            name=f"lt_{c}_{h}")
                            nc.scalar.activation(lt[0:1, :], ctx2[hh][64:65, :],
                                                 LOG)
                            rr = npool.tile([1, QCW], F32, tag="rr",
                                            name=f"rr_{c}_{h}")
                            nc.scalar.activation(rr[0:1, :], lt[0:1, :], EXP,
                                                 scale=-1.0)
                            rb = npool.tile([64, QCW], F32, tag="rb",
                                            name=f"rb_{c}_{h}")
                            nc.gpsimd.partition_broadcast(rb[0:64, :],
                                                          rr[0:1, :])
                            cn = npool.tile([64, QCW], BF16, tag="cn",
                                            name=f"cn_{c}_{h}")
                            nc.vector.tensor_mul(cn[0:64, :], ctx2[hh][0:64, :],
                                                 rb[0:64, :])
                            nc.gpsimd.dma_start(
                                out=ag_in[c, p][64 * hh:64 * (hh + 1), :],
                                in_=cn[0:64, :])

                        nc.gpsimd.collective_compute(
                            "AllGather", mybir.AluOpType.bypass,
                            replica_groups=rg,
                            ins=[ag_in[c, p].opt()], outs=[ag_out[c, p].opt()])

                        # out-projection runs one head-pair behind its
                        # allgather so the PE never stalls on the collective
                        if p == 0 and c > 0:
                            emit_outproj_part(c - 1, 1)
                            emit_outproj_finish(c - 1)
                        elif p == 1:
                            emit_outproj_part(c, 0)

                emit_outproj_part(NQC - 1, 1)
                emit_outproj_finish(NQC - 1)

    nc.compile()
    return nc


_NC_CACHE = None


def _get_nc():
    global _NC_CACHE
    if _NC_CACHE is None:
        _NC_CACHE = build_nc()
    return _NC_CACHE


def _build_in_maps(inputs):
    import ml_dtypes
    x = np.asarray(inputs["x"], dtype=np.float32)
    Wq = np.asarray(inputs["Wq"], dtype=np.float32)
    Wk = np.asarray(inputs["Wk"], dtype=np.float32)
    Wv = np.asarray(inputs["Wv"], dtype=np.float32)
    Wo = np.asarray(inputs["Wo"], dtype=np.float32)
    sp = int(np.asarray(inputs["start_pos"]))

    b, t, d = x.shape
    assert (b, t, d) == (1, T, D), (b, t, d)

    cosT_rep, sinT_rep = _rope_tables(sp)
    xT_r = _round_fp32r(x[0].T)                      # [D, T]
    tri = np.where(np.arange(KBW)[:, None] > np.arange(KBW)[None, :],
                   np.float32(-1e30), np.float32(0.0))
    mask4_np = np.ascontiguousarray(np.tile(tri, (1, 4)))  # [128, 512]
    eye_np = np.tile(np.eye(64, dtype=np.float32), (2, 1))  # [128, 64]

    in_maps = []
    for i in range(N_CORES):
        wqT_i = _round_fp32r(np.ascontiguousarray(Wq[DLOC * i:DLOC * (i + 1), :].T))
        wkv_i = _round_fp32r(np.ascontiguousarray(
            np.concatenate([Wk[HD * i:HD * (i + 1), :].T,
                            Wv[HD * i:HD * (i + 1), :].T], axis=1)))  # [D, 128]
        woT_i = np.ascontiguousarray(
            Wo[DLOC * i:DLOC * (i + 1), :].T).astype(ml_dtypes.bfloat16)
        in_maps.append({
            "xT": xT_r,
            "wqT": wqT_i,
            "wkvT": wkv_i,
            "woT": woT_i,
            "cosT": cosT_rep,
            "sinT": sinT_rep,
            "mask4": mask4_np,
            "eye64": eye_np,
        })
    return in_maps


def kernel(x, Wq, Wk, Wv, Wo, start_pos):
    in_maps = _build_in_maps(dict(x=x, Wq=Wq, Wk=Wk, Wv=Wv, Wo=Wo,
                                  start_pos=start_pos))
    nc = _get_nc()
    res = run_bass_kernel_spmd(nc, in_maps, core_ids=list(range(N_CORES)))

    outT = np.empty((T, D), dtype=np.float32)
    for i in range(N_CORES):
        outT[:, DLOC * i:DLOC * (i + 1)] = res.results[i]["out"].T
    return outT[None, :, :]


if __name__ == "__main__":
    rng = np.random.default_rng(0)
    inputs = {
        "x": rng.standard_normal((1, T, D)).astype(np.float32),
        "Wq": (rng.standard_normal((D, D)) * 0.02).astype(np.float32),
        "Wk": (rng.standard_normal((512, D)) * 0.02).astype(np.float32),
        "Wv": (rng.standard_normal((512, D)) * 0.02).astype(np.float32),
        "Wo": (rng.standard_normal((D, D)) * 0.02).astype(np.float32),
        "start_pos": 0,
    }
    y = kernel(**inputs)
    print("kernel output shape:", y.shape, "finite:", np.isfinite(y).all())
